# revision 1
# baseline (speedup 1.0000x reference)
"""HGAT (GRU + decayed attention + 2x HypergraphConv over 9 hypergraphs) on 8 trn2 cores.

Strategy:
  - Host: reshape/shard only + densify each hypergraph incidence list into a
    dense [1152,1152] operator (bincount), with B^-1/D^-1 scaling baked in,
    shipped bf16 in both row-major layouts (contraction over nodes / edges).
  - Device (SPMD, 8 cores): GRU+attention sharded over nodes (144/core),
    AllGather the attention output, then data-parallel hypergraph convs
    (core c: timestep c; the global-hyp conv is computed redundantly on all
    cores), AllGather timestep conv results + per-timestep sums, final
    temporal attention + output head computed redundantly.
"""
import numpy as np
import ml_dtypes

N, T, H, F_IN, E = 1026, 8, 64, 5, 1026
NP = 1152            # padded N and E (9 * 128)
NCORES = 8
SL = NP // NCORES    # 144 nodes per core
NCH = NP // 128      # 9 contraction chunks
BF = ml_dtypes.bfloat16

_NC_CACHE = {}


# --------------------------------------------------------------------------
# host-side prep
# --------------------------------------------------------------------------

def _densify(idx):
    node = idx[0].astype(np.int64)
    edge = idx[1].astype(np.int64)
    Hm = np.bincount(node * NP + edge, minlength=N * NP).reshape(N, NP)
    Hp = np.zeros((NP, NP), np.float32)
    Hp[:N] = Hm.astype(np.float32)
    degn = Hp.sum(1)
    dege = Hp.sum(0)
    Dinv = np.where(degn > 0, 1.0 / degn, 0.0).astype(np.float32)
    Binv = np.where(dege > 0, 1.0 / dege, 0.0).astype(np.float32)
    HnB = np.ascontiguousarray((Hp * Binv[None, :]).astype(BF))          # [n, e]
    HTeD = np.ascontiguousarray((Hp * Dinv[:, None]).T.astype(BF))       # [e, n]
    return HnB, HTeD


def _host_prep(inp):
    f32 = np.float32
    price = np.asarray(inp["price_input"], f32)          # [N, T, F]
    hyp_T = np.asarray(inp["hyp_T"])                     # [T, 2, nnz]
    hyp = np.asarray(inp["hyp"])                         # [2, nnz]

    WihT = np.ascontiguousarray(np.asarray(inp["Wih"], f32).T)   # [5, 192]
    WhhT = np.ascontiguousarray(np.asarray(inp["Whh"], f32).T)   # [64, 192]
    bih = np.asarray(inp["bih"], f32)
    bhh = np.asarray(inp["bhh"], f32)

    shared = {
        "WihT_rz": np.ascontiguousarray(WihT[:, 0:128]),
        "WihT_n": np.ascontiguousarray(WihT[:, 128:192]),
        "WhhT_rz": np.ascontiguousarray(WhhT[:, 0:128]),
        "WhhT_n": np.ascontiguousarray(WhhT[:, 128:192]),
        "bih_rz": np.ascontiguousarray(bih[0:128, None]),
        "bih_n": np.ascontiguousarray(bih[128:192, None]),
        "bhh_rz": np.ascontiguousarray(bhh[0:128, None]),
        "bhh_n": np.ascontiguousarray(bhh[128:192, None]),
        "Win": np.asarray(inp["Win"], f32),
        "Wout": np.asarray(inp["Wout"], f32),
        "delta": np.ascontiguousarray(
            np.broadcast_to(np.arange(T - 1, -1, -1, dtype=f32), (128, T))),
        "theta1": np.asarray(inp["theta1"], f32),
        "theta2": np.asarray(inp["theta2"], f32),
        "b1": np.ascontiguousarray(np.asarray(inp["bias1"], f32)[:, None]),
        "b2": np.ascontiguousarray(np.asarray(inp["bias2"], f32)[:, None]),
        "w1T": np.ascontiguousarray(np.asarray(inp["w1"], f32).T),   # [7, 64]
        "w2T": np.ascontiguousarray(np.asarray(inp["w2"], f32).T),   # [64, 7]
        "Wl": np.asarray(inp["Wl"], f32),                            # [128, 1]
        "bl_rep": np.full((128, 1), np.asarray(inp["bl"], f32)[0], f32),
        "identF": np.eye(128, dtype=f32),
        "identB": np.eye(128, dtype=BF),
    }

    HnB_G, HTeD_G = _densify(hyp)
    shared["HnB_G"] = HnB_G
    shared["HTeD_G"] = HTeD_G

    price_p = np.zeros((NP, T, F_IN), f32)
    price_p[:N] = price
    ae_p = np.zeros((NP,), f32)
    ae_p[:N] = np.asarray(inp["ae"], f32)[:, 0, 0]
    ab_p = np.zeros((NP,), f32)
    ab_p[:N] = np.asarray(inp["ab"], f32)[:, 0, 0]

    in_maps = []
    for c in range(NCORES):
        sl = slice(c * SL, (c + 1) * SL)
        m = dict(shared)
        m["x5"] = np.ascontiguousarray(
            price_p[sl].transpose(2, 1, 0).reshape(F_IN, T * SL))     # [5, (t n)]
        m["ae_col"] = np.ascontiguousarray(ae_p[sl, None])
        m["ab_col"] = np.ascontiguousarray(ab_p[sl, None])
        HnB_L, HTeD_L = _densify(hyp_T[c])
        m["HnB_L"] = HnB_L
        m["HTeD_L"] = HTeD_L
        in_maps.append(m)
    return in_maps


_IN_SPECS = [
    ("x5", (F_IN, NP), "f32"),
    ("ae_col", (SL, 1), "f32"), ("ab_col", (SL, 1), "f32"),
    ("WihT_rz", (F_IN, 128), "f32"), ("WihT_n", (F_IN, 64), "f32"),
    ("WhhT_rz", (64, 128), "f32"), ("WhhT_n", (64, 64), "f32"),
    ("bih_rz", (128, 1), "f32"), ("bih_n", (64, 1), "f32"),
    ("bhh_rz", (128, 1), "f32"), ("bhh_n", (64, 1), "f32"),
    ("Win", (64, 64), "f32"), ("Wout", (128, 64), "f32"),
    ("delta", (128, T), "f32"),
    ("theta1", (64, 64), "f32"), ("theta2", (64, 64), "f32"),
    ("b1", (64, 1), "f32"), ("b2", (64, 1), "f32"),
    ("w1T", (T - 1, 64), "f32"), ("w2T", (64, T - 1), "f32"),
    ("Wl", (128, 1), "f32"), ("bl_rep", (128, 1), "f32"),
    ("identF", (128, 128), "f32"), ("identB", (128, 128), "bf16"),
    ("HnB_L", (NP, NP), "bf16"), ("HTeD_L", (NP, NP), "bf16"),
    ("HnB_G", (NP, NP), "bf16"), ("HTeD_G", (NP, NP), "bf16"),
]


# --------------------------------------------------------------------------
# device program
# --------------------------------------------------------------------------

def build_program(tc, A, out_ap):
    """Emit the SPMD program. A: dict name -> dram AP. out_ap: [1026,1] f32."""
    import contextlib
    import concourse.bass as bass
    import concourse.mybir as mybir

    nc = tc.nc
    F32 = mybir.dt.float32
    BF16 = mybir.dt.bfloat16
    AF = mybir.ActivationFunctionType
    ALU = mybir.AluOpType
    AX = mybir.AxisListType
    CH3 = ((0, 512), (512, 512), (1024, 128))
    groups = [list(range(NCORES))]

    stack = contextlib.ExitStack()
    CP = stack.enter_context(tc.tile_pool(name="consts", bufs=1))
    WK = stack.enter_context(tc.tile_pool(name="work", bufs=1))
    HP = stack.enter_context(tc.tile_pool(name="hmat", bufs=1))
    DR = stack.enter_context(tc.tile_pool(name="dram", bufs=1, space="DRAM"))

    def load(pool, name, shape, dtype, src_ap):
        t = pool.tile(shape, dtype, name=name)
        nc.sync.dma_start(t[:], src_ap)
        return t

    # ---- small consts ----
    c = {}
    for nm in ("WihT_rz", "WihT_n", "WhhT_rz", "WhhT_n", "bih_rz", "bih_n",
               "bhh_rz", "bhh_n", "Win", "Wout", "delta", "theta1", "theta2",
               "b1", "b2", "w1T", "w2T", "Wl", "bl_rep", "identF", "identB",
               "x5"):
        spec = dict((s[0], s) for s in _IN_SPECS)[nm]
        dt_ = F32 if spec[2] == "f32" else BF16
        c[nm] = load(CP, f"c_{nm}", list(spec[1]), dt_, A[nm][:])
    aeA = load(CP, "aeA", [128, 1], F32, A["ae_col"][0:128])
    aeB = load(CP, "aeB", [16, 1], F32, A["ae_col"][128:144])
    abA = load(CP, "abA", [128, 1], F32, A["ab_col"][0:128])
    abB = load(CP, "abB", [16, 1], F32, A["ab_col"][128:144])

    # ---- H operator tiles (big DMAs) ----
    Hmats = {}
    for nm in ("HnB_L", "HTeD_L", "HnB_G", "HTeD_G"):
        tiles = []
        for k in range(NCH):
            tiles.append(load(HP, f"{nm}_{k}", [128, NP], BF16,
                              A[nm][k * 128:(k + 1) * 128, :]))
        Hmats[nm] = tiles

    identF64 = c["identF"][0:64, 0:64]
    identB64 = c["identB"][0:64, 0:64]

    # ---- persistent work tiles ----
    ctxT = WK.tile([64, T * SL], F32, name="ctxT")          # [h, (t n)]
    ctx_nA = WK.tile([128, T, 64], F32, name="ctx_nA")
    ctx_nB = WK.tile([16, T, 64], F32, name="ctx_nB")
    outT_full = WK.tile([64, NP], F32, name="outT_full")    # gathered attention out
    x1T = WK.tile([64, NP], F32, name="x1T")
    pay = WK.tile([65, NP], F32, name="pay")                # x2T + S row
    combT2 = WK.tile([128, NP], F32, name="combT2")         # [xgT ; xx1T]
    x1gT = WK.tile([64, NP], F32, name="x1gT")

    # ======================= GRU =======================
    with tc.tile_pool(name="sb_gi", bufs=1) as SBGI:
        gi_rz = SBGI.tile([128, T * SL], F32, name="gi_rz")
        gi_n = SBGI.tile([64, T * SL], F32, name="gi_n")
        with tc.tile_pool(name="ps_gi", bufs=1, space="PSUM") as PSGI:
            gi_rz_ps = PSGI.tile([128, T * SL], F32, name="gi_rz_ps", tag="gi")
            for o, w in CH3:
                nc.tensor.matmul(gi_rz_ps[:, o:o + w], c["WihT_rz"][:],
                                 c["x5"][:, o:o + w], start=True, stop=True)
            nc.scalar.activation(gi_rz[:], gi_rz_ps[:], AF.Identity,
                                 bias=c["bih_rz"][:])
            gi_n_ps = PSGI.tile([64, T * SL], F32, name="gi_n_ps", tag="gi")
            for o, w in CH3:
                nc.tensor.matmul(gi_n_ps[:, o:o + w], c["WihT_n"][:],
                                 c["x5"][:, o:o + w], start=True, stop=True)
            nc.scalar.activation(gi_n[:], gi_n_ps[:], AF.Identity,
                                 bias=c["bih_n"][:])

        with tc.tile_pool(name="ps_gru", bufs=1, space="PSUM") as PSG, \
             tc.tile_pool(name="sb_gru", bufs=2) as SBG:
            for t in range(T):
                s = slice(t * SL, (t + 1) * SL)
                sp = slice((t - 1) * SL, t * SL)
                rz = SBG.tile([128, SL], F32, name="rz", tag="rz")
                if t == 0:
                    nc.scalar.activation(rz[:], gi_rz[:, s], AF.Sigmoid,
                                         bias=c["bhh_rz"][:])
                    wn = SBG.tile([64, SL], F32, name="wn", tag="wn")
                    nc.vector.tensor_scalar(wn[:], rz[0:64, :], c["bhh_n"][:],
                                            None, ALU.mult)
                    un = SBG.tile([64, SL], F32, name="un", tag="un")
                    nc.vector.tensor_tensor(un[:], gi_n[:, s], wn[:], ALU.add)
                    nt = SBG.tile([64, SL], F32, name="nt", tag="nt")
                    nc.scalar.activation(nt[:], un[:], AF.Tanh)
                    z0 = SBG.tile([64, SL], F32, name="z0", tag="z0")
                    nc.scalar.activation(z0[:], rz[64:128, :], AF.Copy)
                    mt = SBG.tile([64, SL], F32, name="mt", tag="mt")
                    nc.vector.tensor_tensor(mt[:], nt[:], z0[:], ALU.mult)
                    nc.vector.tensor_tensor(ctxT[:, s], nt[:], mt[:], ALU.subtract)
                else:
                    gh_rz = PSG.tile([128, SL], F32, name="gh_rz", tag="gh_rz")
                    nc.tensor.matmul(gh_rz[:], c["WhhT_rz"][:], ctxT[:, sp],
                                     start=True, stop=True)
                    gh_n = PSG.tile([64, SL], F32, name="gh_n", tag="gh_n")
                    nc.tensor.matmul(gh_n[:], c["WhhT_n"][:], ctxT[:, sp],
                                     start=True, stop=True)
                    urz = SBG.tile([128, SL], F32, name="urz", tag="urz")
                    nc.vector.tensor_tensor(urz[:], gi_rz[:, s], gh_rz[:], ALU.add)
                    nc.scalar.activation(rz[:], urz[:], AF.Sigmoid,
                                         bias=c["bhh_rz"][:])
                    wn = SBG.tile([64, SL], F32, name="wn", tag="wn")
                    nc.vector.scalar_tensor_tensor(wn[:], gh_n[:], c["bhh_n"][:],
                                                   rz[0:64, :], ALU.add, ALU.mult)
                    un = SBG.tile([64, SL], F32, name="un", tag="un")
                    nc.vector.tensor_tensor(un[:], gi_n[:, s], wn[:], ALU.add)
                    nt = SBG.tile([64, SL], F32, name="nt", tag="nt")
                    nc.scalar.activation(nt[:], un[:], AF.Tanh)
                    z0 = SBG.tile([64, SL], F32, name="z0", tag="z0")
                    nc.scalar.activation(z0[:], rz[64:128, :], AF.Copy)
                    dt_ = SBG.tile([64, SL], F32, name="dt_", tag="dt_")
                    nc.vector.tensor_tensor(dt_[:], ctxT[:, sp], nt[:], ALU.subtract)
                    mt = SBG.tile([64, SL], F32, name="mt", tag="mt")
                    nc.vector.tensor_tensor(mt[:], dt_[:], z0[:], ALU.mult)
                    nc.vector.tensor_tensor(ctxT[:, s], mt[:], nt[:], ALU.add)
                # pipelined transpose of this step's h into node-major ctx
                trA = PSG.tile([128, 64], F32, name="trA", tag="trA")
                nc.tensor.transpose(trA[:], ctxT[:, t * SL:t * SL + 128], identF64)
                nc.scalar.activation(ctx_nA[:, t, :], trA[:], AF.Copy)
                trB = PSG.tile([16, 64], F32, name="trB", tag="trB")
                nc.tensor.transpose(trB[:], ctxT[:, t * SL + 128:(t + 1) * SL],
                                    identF64)
                nc.scalar.activation(ctx_nB[:, t, :], trB[:], AF.Copy)

        # ======================= attention =======================
        with tc.tile_pool(name="ps_att", bufs=1, space="PSUM") as PSA, \
             tc.tile_pool(name="sb_att", bufs=1) as SBA:
            lastT = ctxT[:, 7 * SL:8 * SL]
            qT_ps = PSA.tile([64, SL], F32, name="qT_ps", tag="qT")
            nc.tensor.matmul(qT_ps[:], c["Win"][:], lastT, start=True, stop=True)
            combT = SBA.tile([128, SL], F32, name="combT")
            nc.scalar.activation(combT[64:128, :], qT_ps[:], AF.Copy)

            for nm, np_, ctx_n, ae_t, ab_t, csl in (
                    ("A", 128, ctx_nA, aeA, abA, slice(0, 128)),
                    ("B", 16, ctx_nB, aeB, abB, slice(128, SL))):
                q_ps = PSA.tile([np_, 64], F32, name=f"q_ps{nm}", tag=f"q{nm}")
                nc.tensor.matmul(q_ps[:], lastT[:, csl], c["Win"][:],
                                 start=True, stop=True)
                q_s = SBA.tile([np_, 64], F32, name=f"q_s{nm}")
                nc.scalar.activation(q_s[:], q_ps[:], AF.Copy)
                prod = SBA.tile([np_, T, 64], F32, name=f"prod{nm}")
                nc.vector.tensor_tensor(
                    prod[:], ctx_n[:],
                    q_s[:].unsqueeze(1).broadcast_to([np_, T, 64]), ALU.mult)
                sc = SBA.tile([np_, T], F32, name=f"sc{nm}")
                nc.vector.tensor_reduce(sc[:], prod[:], AX.X, ALU.add)
                nm_t = SBA.tile([np_, 1], F32, name=f"nm_t{nm}")
                nc.vector.tensor_reduce(nm_t[:], sc[:], AX.X, ALU.max, negate=True)
                ex = SBA.tile([np_, T], F32, name=f"ex{nm}")
                nc.scalar.activation(ex[:], sc[:], AF.Exp, bias=nm_t[:])
                den = SBA.tile([np_, 1], F32, name=f"den{nm}")
                nc.vector.tensor_reduce(den[:], ex[:], AX.X, ALU.add)
                rcp = SBA.tile([np_, 1], F32, name=f"rcp{nm}")
                nc.vector.reciprocal(rcp[:], den[:])
                wA = SBA.tile([np_, T], F32, name=f"wA{nm}")
                nc.vector.tensor_scalar(wA[:], ex[:], rcp[:], None, ALU.mult)
                nab = SBA.tile([np_, 1], F32, name=f"nab{nm}")
                nc.vector.tensor_scalar(nab[:], ab_t[:], -1.0, None, ALU.mult)
                bt = SBA.tile([np_, T], F32, name=f"bt{nm}")
                nc.scalar.activation(bt[:], c["delta"][0:np_, :], AF.Exp,
                                     scale=nab[:])
                P_t = SBA.tile([np_, T, 64], F32, name=f"P_t{nm}")
                nc.vector.tensor_tensor(
                    P_t[:], ctx_n[:],
                    wA[:].unsqueeze(2).broadcast_to([np_, T, 64]), ALU.mult)
                G_t = SBA.tile([np_, T, 64], F32, name=f"G_t{nm}")
                nc.vector.tensor_tensor(
                    G_t[:], P_t[:],
                    bt[:].unsqueeze(2).broadcast_to([np_, T, 64]), ALU.mult)
                t2_t = SBA.tile([np_, T, 64], F32, name=f"t2_t{nm}")
                nc.scalar.activation(t2_t[:], G_t[:], AF.Relu, scale=ae_t[:])
                sm = SBA.tile([np_, T, 64], F32, name=f"sm{nm}")
                nc.vector.tensor_tensor(sm[:], P_t[:], t2_t[:], ALU.add)
                mixs = SBA.tile([np_, 64], F32, name=f"mixs{nm}")
                nc.vector.tensor_reduce(
                    mixs[:], sm[:].rearrange("p t h -> p h t"), AX.X, ALU.add)
                # transpose mixs into combT rows 0:64
                mtr = PSA.tile([64, np_], F32, name=f"mtr{nm}", tag=f"mtr{nm}")
                nc.tensor.transpose(mtr[:], mixs[:], c["identF"][0:np_, 0:np_])
                nc.scalar.activation(combT[0:64, csl], mtr[:], AF.Copy)

            outT_ps = PSA.tile([64, SL], F32, name="outT_ps", tag="outT")
            nc.tensor.matmul(outT_ps[:], c["Wout"][:], combT[:],
                             start=True, stop=True)
            outT_slice = SBA.tile([64, SL], F32, name="outT_slice")
            nc.scalar.activation(outT_slice[:], outT_ps[:], AF.Tanh)

            # ---- collective 1: allgather attention output ----
            cc1_in = DR.tile([64, SL], F32, name="cc1_in")
            cc1_out = DR.tile([NCORES, 64, SL], F32, name="cc1_out",
                              addr_space="Shared")
            nc.sync.dma_start(cc1_in[:], outT_slice[:])
            nc.gpsimd.collective_compute(
                "AllGather", ALU.bypass, replica_groups=groups,
                ins=[cc1_in[:].opt()], outs=[cc1_out[:].opt()])
            nc.sync.dma_start(
                outT_full[:].rearrange("p (c n) -> p c n", c=NCORES),
                cc1_out[:].rearrange("c p n -> p c n"))

    # ======================= hypergraph convs =======================
    with tc.tile_pool(name="ps_xp", bufs=1, space="PSUM") as PSX, \
         tc.tile_pool(name="ps_acc", bufs=1, space="PSUM") as PAcc, \
         tc.tile_pool(name="sb_conv", bufs=2) as SBC:

        def conv_block(xT_in, theta_t, bcol_t, Hn_ts, HTe_ts, out_dst, tag):
            xp_ps = PSX.tile([128, NCH * 64], F32, name=f"xp_{tag}", tag="xp")
            for k in range(NCH):
                nc.tensor.matmul(xp_ps[:, k * 64:(k + 1) * 64],
                                 xT_in[:, k * 128:(k + 1) * 128], theta_t[:],
                                 start=True, stop=True)
            xpbf = SBC.tile([128, NCH, 64], BF16, name=f"xpbf_{tag}", tag="xpbf")
            nc.scalar.activation(
                xpbf[:], xp_ps[:].rearrange("p (k h) -> p k h", k=NCH), AF.Copy)
            ebT_ps = PAcc.tile([64, NP], F32, name=f"ebT_{tag}", tag="acc")
            for k in range(NCH):
                for o, w in CH3:
                    nc.tensor.matmul(ebT_ps[:, o:o + w], xpbf[:, k, :],
                                     Hn_ts[k][:, o:o + w],
                                     start=(k == 0), stop=(k == NCH - 1))
            ebTbf = SBC.tile([64, NP], BF16, name=f"ebTbf_{tag}", tag="ebTbf")
            nc.vector.tensor_copy(ebTbf[:], ebT_ps[:])
            tr_ps = PSX.tile([128, NCH * 64], BF16, name=f"tr_{tag}", tag="xp")
            for k in range(NCH):
                nc.tensor.transpose(tr_ps[:, k * 64:(k + 1) * 64],
                                    ebTbf[:, k * 128:(k + 1) * 128], identB64)
            ebbf = SBC.tile([128, NCH, 64], BF16, name=f"ebbf_{tag}", tag="ebbf")
            nc.scalar.activation(
                ebbf[:], tr_ps[:].rearrange("p (k h) -> p k h", k=NCH), AF.Copy)
            oT_ps = PAcc.tile([64, NP], F32, name=f"oT_{tag}", tag="acc")
            for k in range(NCH):
                for o, w in CH3:
                    nc.tensor.matmul(oT_ps[:, o:o + w], ebbf[:, k, :],
                                     HTe_ts[k][:, o:o + w],
                                     start=(k == 0), stop=(k == NCH - 1))
            # leaky(o + b) = max(o + b, 0.2 (o + b))
            l1 = SBC.tile([64, NP], F32, name=f"l1_{tag}", tag="lk1")
            nc.vector.tensor_scalar(l1[:], oT_ps[:], bcol_t[:], 0.2,
                                    ALU.add, ALU.mult)
            l2 = SBC.tile([64, NP], F32, name=f"l2_{tag}", tag="lk2")
            nc.scalar.activation(l2[:], oT_ps[:], AF.Identity, bias=bcol_t[:])
            nc.vector.tensor_tensor(out_dst, l2[:], l1[:], ALU.max)

        conv_block(outT_full[:], c["theta1"], c["b1"], Hmats["HnB_L"],
                   Hmats["HTeD_L"], x1T[:], "L1")
        conv_block(outT_full[:], c["theta1"], c["b1"], Hmats["HnB_G"],
                   Hmats["HTeD_G"], x1gT[:], "G1")
        conv_block(x1T[:], c["theta2"], c["b2"], Hmats["HnB_L"],
                   Hmats["HTeD_L"], pay[0:64, :], "L2")
        conv_block(x1gT[:], c["theta2"], c["b2"], Hmats["HnB_G"],
                   Hmats["HTeD_G"], combT2[0:64, :], "G2")

        # per-timestep sum S into payload row 64
        nc.vector.memset(pay[64:65, :], 0.0)
        S_col = SBC.tile([64, 1], F32, name="S_col", tag="scol")
        nc.vector.tensor_reduce(S_col[:], pay[0:64, 0:N], AX.X, ALU.add)
        S_tr = PSX.tile([1, 64], F32, name="S_tr", tag="str")
        nc.tensor.transpose(S_tr[:], S_col[:], identF64)
        nc.scalar.activation(pay[64:65, 0:64], S_tr[:], AF.Copy)

        # ---- collective 2: allgather conv results + sums ----
        cc2_in = DR.tile([65, NP], F32, name="cc2_in")
        cc2_out = DR.tile([NCORES, 65, NP], F32, name="cc2_out",
                          addr_space="Shared")
        nc.sync.dma_start(cc2_in[:], pay[:])
        nc.gpsimd.collective_compute(
            "AllGather", ALU.bypass, replica_groups=groups,
            ins=[cc2_in[:].opt()], outs=[cc2_out[:].opt()])

        # ======================= final stage =======================
        with tc.tile_pool(name="sb_fin", bufs=1) as SBF:
            x2 = []
            for t_ in range(4):
                xt_ = SBF.tile([64, NP], F32, name=f"x2_{t_}")
                nc.sync.dma_start(xt_[:], cc2_out[t_, 0:64, :])
                x2.append(xt_)
            Spart = SBF.tile([T - 1, 64], F32, name="Spart")
            nc.sync.dma_start(Spart[:], cc2_out[0:7, 64, 0:64])
            Spart1 = SBF.tile([T - 1, 64], F32, name="Spart1")
            nc.sync.dma_start(Spart1[:], cc2_out[1:8, 64, 0:64])
            Ssum0 = SBF.tile([T - 1, 1], F32, name="Ssum0")
            nc.vector.tensor_reduce(Ssum0[:], Spart[:], AX.X, ALU.add)
            Ssum1 = SBF.tile([T - 1, 1], F32, name="Ssum1")
            nc.vector.tensor_reduce(Ssum1[:], Spart1[:], AX.X, ALU.add)
            zv = SBF.tile([T - 1, 1], F32, name="zv")
            nc.vector.tensor_tensor(zv[:], Ssum1[:], Ssum0[:], ALU.subtract)
            y_ps = PSX.tile([64, 1], F32, name="y_ps", tag="str")
            nc.tensor.matmul(y_ps[:], c["w1T"][:], zv[:], start=True, stop=True)
            y1 = SBF.tile([64, 1], F32, name="y1")
            nc.vector.tensor_scalar(y1[:], y_ps[:], 0.2, None, ALU.mult)
            y_s = SBF.tile([64, 1], F32, name="y_s")
            nc.vector.tensor_tensor(y_s[:], y_ps[:], y1[:], ALU.max)
            wat_ps = PSX.tile([T - 1, 1], F32, name="wat_ps", tag="str")
            nc.tensor.matmul(wat_ps[:], c["w2T"][:], y_s[:], start=True, stop=True)
            wat_s = SBF.tile([T - 1, 1], F32, name="wat_s")
            nc.scalar.activation(wat_s[:], wat_ps[:], AF.Copy)
            watT_ps = PSX.tile([1, T - 1], F32, name="watT_ps", tag="str")
            nc.tensor.transpose(watT_ps[:], wat_s[:], c["identF"][0:7, 0:7])
            watT = SBF.tile([1, T - 1], F32, name="watT")
            nc.scalar.activation(watT[:], watT_ps[:], AF.Copy)
            nmw = SBF.tile([1, 1], F32, name="nmw")
            nc.vector.tensor_reduce(nmw[:], watT[:], AX.X, ALU.max, negate=True)
            exw = SBF.tile([1, T - 1], F32, name="exw")
            nc.scalar.activation(exw[:], watT[:], AF.Exp, bias=nmw[:])
            denw = SBF.tile([1, 1], F32, name="denw")
            nc.vector.tensor_reduce(denw[:], exw[:], AX.X, ALU.add)
            rw = SBF.tile([1, 1], F32, name="rw")
            nc.vector.reciprocal(rw[:], denw[:])
            wsm = SBF.tile([1, T - 1], F32, name="wsm")
            nc.vector.tensor_scalar(wsm[:], exw[:], rw[:], None, ALU.mult)
            # broadcast wsm[0]/wsm[2] to 64 partitions via DRAM bounce
            wb = DR.tile([1, T - 1], F32, name="wb")
            nc.sync.dma_start(wb[:], wsm[:])
            w0b = SBF.tile([64, 1], F32, name="w0b")
            nc.sync.dma_start(w0b[:], wb[0:1, 0:1].broadcast_to([64, 1]))
            w2b = SBF.tile([64, 1], F32, name="w2b")
            nc.sync.dma_start(w2b[:], wb[0:1, 2:3].broadcast_to([64, 1]))
            d0 = SBF.tile([64, NP], F32, name="d0")
            nc.vector.tensor_tensor(d0[:], x2[1][:], x2[0][:], ALU.subtract)
            d2 = SBF.tile([64, NP], F32, name="d2")
            nc.vector.tensor_tensor(d2[:], x2[3][:], x2[2][:], ALU.subtract)
            m0 = SBF.tile([64, NP], F32, name="m0")
            nc.vector.tensor_scalar(m0[:], d0[:], w0b[:], None, ALU.mult)
            nc.vector.scalar_tensor_tensor(combT2[64:128, :], d2[:], w2b[:],
                                           m0[:], ALU.mult, ALU.add)
            # output head: res[n] = leaky(Wl . comb[:, n] + bl)
            res_ps = PSX.tile([128, NCH], F32, name="res_ps", tag="str")
            for k in range(NCH):
                nc.tensor.matmul(res_ps[:, k:k + 1],
                                 combT2[:, k * 128:(k + 1) * 128], c["Wl"][:],
                                 start=True, stop=True)
            r1 = SBF.tile([128, NCH], F32, name="r1")
            nc.vector.tensor_scalar(r1[:], res_ps[:], c["bl_rep"][:], 0.2,
                                    ALU.add, ALU.mult)
            r2 = SBF.tile([128, NCH], F32, name="r2")
            nc.vector.tensor_scalar(r2[:], res_ps[:], c["bl_rep"][:], None,
                                    ALU.add)
            res_s = SBF.tile([128, NCH], F32, name="res_s")
            nc.vector.tensor_tensor(res_s[:], r2[:], r1[:], ALU.max)
            nc.sync.dma_start(
                out_ap[0:1024, 0:1].rearrange("(k p) o -> p k o", p=128),
                res_s[:, 0:8].unsqueeze(2))
            nc.sync.dma_start(out_ap[1024:1026, 0:1], res_s[0:2, 8:9])

    stack.close()


# --------------------------------------------------------------------------
# entry points
# --------------------------------------------------------------------------

def _make_nc():
    if "nc" in _NC_CACHE:
        return _NC_CACHE["nc"]
    import concourse.bacc as bacc
    import concourse.mybir as mybir
    from concourse import tile

    nc = bacc.Bacc("TRN2", target_bir_lowering=False, debug=False,
                   enable_asserts=True, num_devices=NCORES)
    A = {}
    for nm, shape, dt_ in _IN_SPECS:
        A[nm] = nc.dram_tensor(
            nm, list(shape),
            mybir.dt.float32 if dt_ == "f32" else mybir.dt.bfloat16,
            kind="ExternalInput").ap()
    out_h = nc.dram_tensor("out", [N, 1], mybir.dt.float32,
                           kind="ExternalOutput")
    with tile.TileContext(nc) as tc:
        build_program(tc, A, out_h.ap())
    nc.compile()
    _NC_CACHE["nc"] = nc
    return nc


def kernel(**inputs):
    from concourse.bass_utils import run_bass_kernel_spmd
    nc = _make_nc()
    in_maps = _host_prep(inputs)
    res = run_bass_kernel_spmd(nc, in_maps, list(range(NCORES)))
    return np.asarray(res.results[0]["out"])

